# revision 1
# baseline (speedup 1.0000x reference)
"""AttentionMemory kernel for Trainium2 (8 NeuronCores, Bass/Tile).

Reference computation (per batch b):
    affinity[n, m] = (2 * mk[:,n]@qk[:,m] - ||mk[:,n]||^2 - ||qk[:,m]||^2) / 8
    out[n, m]      = softmax over n (memory axis)

Softmax over n is invariant to per-column constants, so the -||qk_m||^2
term is dropped.  Logits are produced by an augmented matmul:
    lhsT (stationary) = [0.25 * qk ; -0.125]          -> [65, Mc]
    rhs  (moving)     = [mk        ; a_n   ]          -> [65, N]
    psum[m, n]        = 0.25*dot(qk_m, mk_n) - 0.125*a_n   == logits[m, n]
with a_n = sum_c mk[c,n]^2 precomputed on the host.

Precision: inputs are split hi/lo into bf16 pairs on the host and each
logit tile accumulates three bf16 matmuls in PSUM
    qh@mh + qh@ml + ql@mh      (ql@ml dropped, ~6e-5 logit error)
giving ~1e-4 relative output error at full 1-cycle/row PE throughput
(plain fp32 matmul is 4x slower; float32r is fast but tf32-precision).

Sharding: core c handles batch c//2, query-column half c%2 (communication
free: softmax is over the full n axis which each core holds).  Each core
writes out_c[m, n]; the host transposes to the reference [n, m] layout.

Input DRAM layout is packed by first-use so the head of the pipeline
starts as early as possible:
    q2 [65, 16*252]: per m-strip s, block [qh_s (126) | ql_s (126)]
    m2 [65,  8*1008]: per n-chunk c, block [mh_c (504) | ml_c (504)]

Logits are <= 0, so exp() never overflows and the max-subtraction pass is
skipped (min logit ~ -35 -> exp ~ 1e-16, no underflow in fp32).

Per-core roofline: 32.5 MB f32 output at ~360 GB/s ~= 90 us.  Pipeline:
PE (bf16 matmuls) -> ACT (exp + fused row-sum, PSUM->SBUF) -> DVE
(reciprocal + normalize) -> HWDGE store; the store stream runs gap-free.
"""

import numpy as np

B, CK, H, W = 4, 64, 48, 84
N = H * W            # 4032 memory pixels (softmax axis)
HALF = N // 2        # 2016 query pixels per core
M_STRIP = 126        # output-partition strip size (16 * 126 = 2016)
N_STRIPS = HALF // M_STRIP
K_AUG = CK + 1       # 65: contraction dim incl. the -a_n row

N_QUARTER = N // 4   # 1008: one PSUM tile (2 banks) / one ACT exp call
N_CHUNK = 504        # matmul moving free dim (<=512, one PSUM bank)
N_CHUNKS = N // N_CHUNK  # 8

_CACHE = {}


def _build_nc():
    import concourse.bacc as bacc
    import concourse.mybir as mybir
    import concourse.tile as tile

    f32 = mybir.dt.float32
    bf16 = mybir.dt.bfloat16
    Exp = mybir.ActivationFunctionType.Exp

    nc = bacc.Bacc("TRN2", target_bir_lowering=False, debug=False)

    q2_d = nc.dram_tensor("q2", [K_AUG, 2 * HALF], bf16, kind="ExternalInput")
    m2_d = nc.dram_tensor("m2", [K_AUG, 2 * N], bf16, kind="ExternalInput")
    out_d = nc.dram_tensor("out_c", [HALF, N], f32, kind="ExternalOutput")

    with tile.TileContext(nc) as tc:
        with (
            tc.tile_pool(name="singles", bufs=1) as singles,
            tc.tile_pool(name="psum", bufs=4, space="PSUM") as psum_pool,
            tc.tile_pool(name="exp", bufs=3) as exp_pool,
            tc.tile_pool(name="outs", bufs=4) as out_pool,
            tc.tile_pool(name="stats", bufs=8) as stats_pool,
        ):
            # --- prewarm: ACT exp table load + PE HAM spin-up during the
            # input DMAs -----------------------------------------------------
            wtab = singles.tile([1, 2], f32)
            nc.vector.memset(wtab, 0.0)
            nc.scalar.activation(wtab[:, 1:2], wtab[:, 0:1], Exp)
            wsrc = singles.tile([K_AUG, 256], bf16)
            nc.vector.memset(wsrc, 0.0)
            wps = psum_pool.tile([M_STRIP, 256], f32, tag="ps")
            for _ in range(12):
                nc.tensor.matmul(
                    wps, wsrc[:, :M_STRIP], wsrc, start=True, stop=True
                )

            # --- inputs, staged by first use.  q2 rides the ACT HWDGE ring,
            # m2 the SP ring, so their dispatches overlap ---------------------
            q2_s = singles.tile([K_AUG, 2 * HALF], bf16)
            m2_s = singles.tile([K_AUG, 2 * N], bf16)
            nc.scalar.dma_start(out=q2_s[:, :252], in_=q2_d[:, :252])
            for c0, c1 in ((0, 2), (2, 4), (4, 6), (6, 8)):
                sl = slice(c0 * 1008, c1 * 1008)
                nc.sync.dma_start(out=m2_s[:, sl], in_=m2_d[:, sl])
            nc.sync.dma_start(out=q2_s[:, 252:], in_=q2_d[:, 252:])

            def mh(c):  # rhs hi slice for n-chunk c
                return m2_s[:, c * 1008 : c * 1008 + N_CHUNK]

            def ml(c):  # rhs lo slice for n-chunk c
                return m2_s[:, c * 1008 + N_CHUNK : (c + 1) * 1008]

            for s in range(N_STRIPS):
                m0 = s * M_STRIP
                qh_l = q2_s[:, s * 252 : s * 252 + M_STRIP]
                ql_l = q2_s[:, s * 252 + M_STRIP : (s + 1) * 252]

                exp_t = exp_pool.tile([M_STRIP, N], f32, tag="exp")
                acc = stats_pool.tile([M_STRIP, 8], f32, tag="acc")

                # ACT pieces = pairs of 504-wide n-chunks (one 2-bank PSUM
                # tile / one exp call each)
                pieces = [[0, 1], [2, 3], [4, 5], [6, 7]]
                for pi, piece in enumerate(pieces):
                    k = len(piece)
                    # one PSUM bank (512 cols) per 504-wide chunk; each chunk
                    # starts on a bank boundary — PE writes must not straddle
                    # a bank
                    ps = psum_pool.tile([M_STRIP, 512 * k], f32, tag="ps")
                    for cc, c in enumerate(piece):
                        psl = ps[:, cc * 512 : cc * 512 + N_CHUNK]
                        nc.tensor.matmul(psl, qh_l, mh(c), start=True, stop=False)
                        nc.tensor.matmul(psl, qh_l, ml(c), start=False, stop=False)
                        nc.tensor.matmul(psl, ql_l, mh(c), start=False, stop=True)
                    # exp(logits) PSUM->SBUF with fused per-partition row sum;
                    # the strided 3D views skip the 8 pad columns per bank
                    e0 = piece[0] * N_CHUNK
                    nc.scalar.activation(
                        exp_t[:, e0 : e0 + k * N_CHUNK].rearrange(
                            "p (b c) -> p b c", b=k
                        ),
                        ps.rearrange("p (b c) -> p b c", b=k)[:, :, :N_CHUNK],
                        Exp,
                        accum_out=acc[:, pi : pi + 1],
                    )

                ssum = stats_pool.tile([M_STRIP, 1], f32, tag="ssum")
                nc.vector.reduce_sum(
                    ssum, acc[:, : len(pieces)], axis=mybir.AxisListType.X
                )
                rcp = stats_pool.tile([M_STRIP, 1], f32, tag="rcp")
                nc.vector.reciprocal(rcp, ssum)

                # strip 0 stores in quarters to start the store stream early;
                # steady state stores in 1 MB halves (better real-HW DMA
                # efficiency at equal modeled time)
                out_t = out_pool.tile([M_STRIP, N], f32, tag="out")
                if s == 0:
                    bounds = [0, 1008, 2016, 3024, N]
                else:
                    bounds = [0, N // 2, N]
                for p0, p1 in zip(bounds, bounds[1:]):
                    sl = slice(p0, p1)
                    nc.vector.tensor_scalar_mul(out_t[:, sl], exp_t[:, sl], rcp)
                    nc.sync.dma_start(
                        out=out_d[m0 : m0 + M_STRIP, sl], in_=out_t[:, sl]
                    )

    nc.compile()
    return nc


def _get_nc():
    if "nc" not in _CACHE:
        _CACHE["nc"] = _build_nc()
    return _CACHE["nc"]


def _split_bf16(x: np.ndarray):
    """x (f32) -> (hi, lo) bf16 with hi + lo ~= x (~16 mantissa bits)."""
    import ml_dtypes

    hi = x.astype(ml_dtypes.bfloat16)
    lo = (x - hi.astype(np.float32)).astype(ml_dtypes.bfloat16)
    return hi, lo


def kernel(mk: np.ndarray, qk: np.ndarray) -> np.ndarray:
    import ml_dtypes
    from concourse import bass_utils

    mk = np.asarray(mk, dtype=np.float32).reshape(B, CK, N)
    qk = np.asarray(qk, dtype=np.float32).reshape(B, CK, N)
    a = np.einsum("bcn,bcn->bn", mk, mk)  # sum_c mk^2, [B, N]

    in_maps = []
    for core in range(8):
        b, h = divmod(core, 2)
        mk_aug = np.empty((K_AUG, N), np.float32)
        mk_aug[:CK] = mk[b]
        mk_aug[CK] = a[b]
        mh, ml = _split_bf16(mk_aug)
        # chunk-pair packed: block c = [mh_c | ml_c], each N_CHUNK wide
        m2 = np.empty((K_AUG, 2 * N), ml_dtypes.bfloat16)
        m3 = m2.reshape(K_AUG, N_CHUNKS, 2, N_CHUNK)
        m3[:, :, 0] = mh.reshape(K_AUG, N_CHUNKS, N_CHUNK)
        m3[:, :, 1] = ml.reshape(K_AUG, N_CHUNKS, N_CHUNK)

        qk_aug = np.empty((K_AUG, HALF), np.float32)
        qk_aug[:CK] = 0.25 * qk[b, :, h * HALF : (h + 1) * HALF]
        qk_aug[CK] = -0.125
        qh, ql = _split_bf16(qk_aug)
        ql[CK] = 0  # a_n row must enter exactly once (via qh row 64)
        # strip packed: block s = [qh_s | ql_s], each M_STRIP wide
        q2 = np.empty((K_AUG, 2 * HALF), ml_dtypes.bfloat16)
        q3 = q2.reshape(K_AUG, N_STRIPS, 2, M_STRIP)
        q3[:, :, 0] = qh.reshape(K_AUG, N_STRIPS, M_STRIP)
        q3[:, :, 1] = ql.reshape(K_AUG, N_STRIPS, M_STRIP)

        in_maps.append({"q2": q2, "m2": m2})

    res = bass_utils.run_bass_kernel_spmd(
        _get_nc(), in_maps, core_ids=list(range(8))
    )
    _CACHE["last_results"] = res

    out = np.empty((B, N, N), np.float32)
    for core in range(8):
        b, h = divmod(core, 2)
        out[b, :, h * HALF : (h + 1) * HALF] = res.results[core]["out_c"].T
    return out



# revision 3
# speedup vs baseline: 1.2845x; 1.2845x over previous
"""AttentionMemory kernel for Trainium2 (8 NeuronCores, Bass/Tile).

Reference computation (per batch b):
    affinity[n, m] = (2 * mk[:,n]@qk[:,m] - ||mk[:,n]||^2 - ||qk[:,m]||^2) / 8
    out[n, m]      = softmax over n (memory axis)

Logits come from a single float32r (tf32-speed) augmented matmul:
    lhsT (stationary) = [0.25 * qk ; -0.125]      -> [65, Mc]
    rhs  (moving)     = [mk        ; a_n - abar]  -> [65, N]
    psum[m, n]        = 0.25*qk_m.mk_n - 0.125*(a_n - abar)
with a_n = sum_c mk[c,n]^2 precomputed on the host.  The ACT exp pass adds a
per-partition bias -0.125*(||qk_m||^2 + abar), making the exp argument exactly
-||mk_n - qk_m||^2 / 8 <= 0: no overflow, and per-column constants cancel in
the softmax.  float32r rounds inputs to ~tf32; simulated end-to-end absmax
error is ~2.3e-3 of scale (gate 2e-2).

exp values and the normalized output are staged in fp16 (DVE runs 4x in
16-bit, DMA stores are half-size); the host upconverts to f32 during the
gather/transpose.  Row sums ride the ACT accumulator (free), reciprocal +
scale on DVE.

Sharding: core c handles batch c//2, query-column half c%2 (communication
free: softmax is over the full n axis which each core holds).  Each core
writes out_c[m, n] fp16; the host transposes to the reference [n, m] layout.

Per-core budget (TimelineSim cost model): ACT exp 0.833ns/col * 64512 cols
~= 60us (bottleneck), fp16 stores 45us, PE f32r 27-54us, DVE ~39us.
"""

import numpy as np

B, CK, H, W = 4, 64, 48, 84
N = H * W            # 4032 memory pixels (softmax axis)
HALF = N // 2        # 2016 query pixels per core
M_STRIP = 126        # output-partition strip size (16 * 126 = 2016)
N_STRIPS = HALF // M_STRIP
K_AUG = CK + 1       # 65: contraction dim incl. the (a_n - abar) row

N_CHUNK = 504        # matmul moving free dim; 4 chunks per 4-bank PSUM piece
PIECE = 4 * N_CHUNK  # 2016 cols per ACT exp call
N_PIECES = N // PIECE  # 2

_CACHE = {}


def _build_nc():
    import concourse.bacc as bacc
    import concourse.mybir as mybir
    import concourse.tile as tile

    f32 = mybir.dt.float32
    f32r = mybir.dt.float32r
    f16 = mybir.dt.float16
    Exp = mybir.ActivationFunctionType.Exp

    nc = bacc.Bacc("TRN2", target_bir_lowering=False, debug=False)

    q_d = nc.dram_tensor("q2", [K_AUG, HALF], f32r, kind="ExternalInput")
    m_d = nc.dram_tensor("m2", [K_AUG, N], f32r, kind="ExternalInput")
    b_d = nc.dram_tensor("bias", [M_STRIP, N_STRIPS], f32, kind="ExternalInput")
    out_d = nc.dram_tensor("out_c", [HALF, N], f16, kind="ExternalOutput")

    with tile.TileContext(nc) as tc:
        with (
            tc.tile_pool(name="singles", bufs=1) as singles,
            tc.tile_pool(name="psum", bufs=2, space="PSUM") as psum_pool,
            tc.tile_pool(name="exp", bufs=3) as exp_pool,
            tc.tile_pool(name="outs", bufs=3) as out_pool,
            tc.tile_pool(name="stats", bufs=8) as stats_pool,
        ):
            # --- prewarm: ACT exp table load + PE pstate ramp during the
            # input DMAs -----------------------------------------------------
            wtab = singles.tile([1, 2], f32)
            nc.vector.memset(wtab, 0.0)
            nc.scalar.activation(wtab[:, 1:2], wtab[:, 0:1], Exp)
            wsrc = singles.tile([K_AUG, 256], f32r)
            nc.vector.memset(wsrc, 0.0)
            wps = psum_pool.tile([M_STRIP, 2048], f32, tag="ps")
            for _ in range(14):
                nc.tensor.matmul(
                    wps[:, :256], wsrc[:, :M_STRIP], wsrc, start=True, stop=True
                )

            # --- inputs, staged by first use.  SP ring: first q strip, then
            # m chunks; ACT ring: rest of q + bias ---------------------------
            q_s = singles.tile([K_AUG, HALF], f32r)
            m_s = singles.tile([K_AUG, N], f32r)
            b_s = singles.tile([M_STRIP, N_STRIPS], f32)
            nc.sync.dma_start(out=q_s[:, :M_STRIP], in_=q_d[:, :M_STRIP])
            for c in range(8):
                sl = slice(c * N_CHUNK, (c + 1) * N_CHUNK)
                nc.sync.dma_start(out=m_s[:, sl], in_=m_d[:, sl])
            nc.scalar.dma_start(out=b_s, in_=b_d[:, :])
            nc.scalar.dma_start(out=q_s[:, M_STRIP:], in_=q_d[:, M_STRIP:])

            for s in range(N_STRIPS):
                m0 = s * M_STRIP
                q_l = q_s[:, m0 : m0 + M_STRIP]

                exp_t = exp_pool.tile([M_STRIP, N], f16, tag="exp")
                acc = stats_pool.tile([M_STRIP, 2], f32, tag="acc")

                for p in range(N_PIECES):
                    # 4 PSUM banks per piece; each 504-col chunk starts on a
                    # bank boundary (512 f32) so PE writes never straddle one
                    ps = psum_pool.tile([M_STRIP, 2048], f32, tag="ps")
                    for cc in range(4):
                        c = 4 * p + cc
                        nc.tensor.matmul(
                            ps[:, cc * 512 : cc * 512 + N_CHUNK],
                            q_l,
                            m_s[:, c * N_CHUNK : (c + 1) * N_CHUNK],
                            start=True,
                            stop=True,
                        )
                    # exp(logits + bias_m) PSUM->SBUF fp16 with fused
                    # per-partition row sum; 3D views skip the 8 pad cols/bank
                    e0 = p * PIECE
                    nc.scalar.activation(
                        exp_t[:, e0 : e0 + PIECE].rearrange(
                            "p (b c) -> p b c", b=4
                        ),
                        ps.rearrange("p (b c) -> p b c", b=4)[:, :, :N_CHUNK],
                        Exp,
                        bias=b_s[:, s : s + 1],
                        accum_out=acc[:, p : p + 1],
                    )

                ssum = stats_pool.tile([M_STRIP, 1], f32, tag="ssum")
                nc.vector.reduce_sum(ssum, acc, axis=mybir.AxisListType.X)
                rcp = stats_pool.tile([M_STRIP, 1], f32, tag="rcp")
                nc.vector.reciprocal(rcp, ssum)

                out_t = out_pool.tile([M_STRIP, N], f16, tag="out")
                nc.vector.tensor_scalar_mul(out_t, exp_t, rcp)
                # halved stores keep the store stream fine-grained; strip 0
                # quarters so the first bytes hit the DMA ring early
                bounds = (
                    [0, 1008, 2016, 3024, N] if s == 0 else [0, N // 2, N]
                )
                for p0, p1 in zip(bounds, bounds[1:]):
                    nc.sync.dma_start(
                        out=out_d[m0 : m0 + M_STRIP, p0:p1],
                        in_=out_t[:, p0:p1],
                    )

    nc.compile()
    return nc


def _get_nc():
    if "nc" not in _CACHE:
        _CACHE["nc"] = _build_nc()
    return _CACHE["nc"]


def kernel(mk: np.ndarray, qk: np.ndarray) -> np.ndarray:
    from concourse import bass_utils

    mk = np.asarray(mk, dtype=np.float32).reshape(B, CK, N)
    qk = np.asarray(qk, dtype=np.float32).reshape(B, CK, N)
    a = np.einsum("bcn,bcn->bn", mk, mk)        # ||mk_n||^2, [B, N]
    cq = np.einsum("bcm,bcm->bm", qk, qk)       # ||qk_m||^2, [B, M]
    abar = a.mean(axis=1)                       # [B]

    in_maps = []
    for core in range(8):
        b, h = divmod(core, 2)
        m2 = np.empty((K_AUG, N), np.float32)
        m2[:CK] = mk[b]
        m2[CK] = a[b] - abar[b]

        q2 = np.empty((K_AUG, HALF), np.float32)
        q2[:CK] = 0.25 * qk[b, :, h * HALF : (h + 1) * HALF]
        q2[CK] = -0.125

        bias = (
            (-0.125 * (cq[b, h * HALF : (h + 1) * HALF] + abar[b]))
            .astype(np.float32)
            .reshape(N_STRIPS, M_STRIP)
            .T.copy()
        )
        in_maps.append({"q2": q2, "m2": m2, "bias": bias})

    res = bass_utils.run_bass_kernel_spmd(
        _get_nc(), in_maps, core_ids=list(range(8))
    )
    _CACHE["last_results"] = res

    out = np.empty((B, N, N), np.float32)
    for core in range(8):
        b, h = divmod(core, 2)
        out[b, :, h * HALF : (h + 1) * HALF] = (
            res.results[core]["out_c"].T.astype(np.float32)
        )
    return out


# revision 28
# speedup vs baseline: 1.3195x; 1.0272x over previous
"""AttentionMemory kernel for Trainium2 (8 NeuronCores, Bass/Tile).

Reference computation (per batch b):
    affinity[n, m] = (2 * mk[:,n]@qk[:,m] - ||mk[:,n]||^2 - ||qk[:,m]||^2) / 8
    out[n, m]      = softmax over n (memory axis)

Logits come from a single float32r (tf32-speed) augmented matmul:
    lhsT (stationary) = [0.25 * qk ; -0.125]      -> [65, Mc]
    rhs  (moving)     = [mk        ; a_n - abar]  -> [65, N]
    psum[m, n]        = 0.25*qk_m.mk_n - 0.125*(a_n - abar)
with a_n = sum_c mk[c,n]^2 precomputed on the host.  The ACT exp pass adds a
per-partition bias -0.125*(||qk_m||^2 + abar), making the exp argument exactly
-||mk_n - qk_m||^2 / 8 <= 0: no overflow, and per-column constants cancel in
the softmax.  float32r rounds inputs to ~tf32; simulated end-to-end absmax
error is ~2.3e-3 of scale (gate 2e-2).

exp values and the normalized output are staged in fp16 (DVE runs 4x in
16-bit, DMA stores are half-size); the host upconverts to f32 during the
gather/transpose.  Row sums ride the ACT accumulator (free), reciprocal +
scale on DVE.

Sharding: core c handles batch c//2, query-column half c%2 (communication
free: softmax is over the full n axis which each core holds).  Each core
writes out_c[m, n] fp16; the host transposes to the reference [n, m] layout.

Per-core budget (TimelineSim cost model): ACT exp 0.833ns/col * 64512 cols
~= 60us (bottleneck), fp16 stores 45us, PE f32r 27-54us, DVE ~39us.
"""

import numpy as np

B, CK, H, W = 4, 64, 48, 84
N = H * W            # 4032 memory pixels (softmax axis)
HALF = N // 2        # 2016 query pixels per core
M_STRIP = 126        # output-partition strip size (16 * 126 = 2016)
N_STRIPS = HALF // M_STRIP
K_AUG = CK + 1       # 65: contraction dim incl. the (a_n - abar) row

N_CHUNK = 504        # matmul moving free dim; 4 chunks per 4-bank PSUM piece
PIECE = 4 * N_CHUNK  # 2016 cols per ACT exp call
N_PIECES = N // PIECE  # 2

_CACHE = {}


def _build_nc():
    import concourse.bacc as bacc
    import concourse.mybir as mybir
    import concourse.tile as tile

    f32 = mybir.dt.float32
    f32r = mybir.dt.float32r
    bf16 = mybir.dt.bfloat16
    f16 = mybir.dt.float16
    Exp = mybir.ActivationFunctionType.Exp

    nc = bacc.Bacc("TRN2", target_bir_lowering=False, debug=False)

    q_d = nc.dram_tensor("q2", [K_AUG, HALF], f32r, kind="ExternalInput")
    m_d = nc.dram_tensor("m2", [K_AUG, N], f32r, kind="ExternalInput")
    b_d = nc.dram_tensor("bias", [M_STRIP, N_STRIPS], f32, kind="ExternalInput")
    out_d = nc.dram_tensor("out_c", [HALF, N], f16, kind="ExternalOutput")

    with tile.TileContext(nc) as tc:
        with (
            tc.tile_pool(name="singles", bufs=1) as singles,
            tc.tile_pool(name="psum", bufs=2, space="PSUM") as psum_pool,
            tc.tile_pool(name="exp", bufs=3) as exp_pool,
            tc.tile_pool(name="outs", bufs=3) as out_pool,
            tc.tile_pool(name="stats", bufs=8) as stats_pool,
        ):
            # --- prewarm: ACT exp table load + PE pstate ramp during the
            # input DMAs -----------------------------------------------------
            wtab = singles.tile([1, 2], f32)
            nc.vector.memset(wtab, 0.0)
            nc.scalar.activation(wtab[:, 1:2], wtab[:, 0:1], Exp)
            wsrc = singles.tile([K_AUG, 256], bf16)
            nc.vector.memset(wsrc, 0.0)
            wps = psum_pool.tile([M_STRIP, 2048], f32, tag="ps")
            for _ in range(14):
                nc.tensor.matmul(
                    wps[:, :256], wsrc[:, :M_STRIP], wsrc, start=True, stop=True
                )

            # --- inputs, staged by first use.  SP ring: bias (tiny, gates the
            # first exp) + strip-0 q + first two m chunks + rest of q; Pool
            # ring (SWDGE, otherwise idle) carries the remaining m chunks so
            # the two sequencers dispatch concurrently and the ACT sequencer
            # stays free for exp dispatches --------------------------------
            q_s = singles.tile([K_AUG, HALF], f32r)
            m_s = singles.tile([K_AUG, N], f32r)
            b_s = singles.tile([M_STRIP, N_STRIPS], f32)
            nc.sync.dma_start(out=b_s, in_=b_d[:, :])
            nc.sync.dma_start(out=q_s[:, :M_STRIP], in_=q_d[:, :M_STRIP])
            for c in range(2):
                sl = slice(c * N_CHUNK, (c + 1) * N_CHUNK)
                nc.sync.dma_start(out=m_s[:, sl], in_=m_d[:, sl])
            for c in range(2, 8):
                sl = slice(c * N_CHUNK, (c + 1) * N_CHUNK)
                nc.gpsimd.dma_start(out=m_s[:, sl], in_=m_d[:, sl])
            nc.scalar.dma_start(out=q_s[:, M_STRIP:], in_=q_d[:, M_STRIP:])

            for s in range(N_STRIPS):
                m0 = s * M_STRIP
                q_l = q_s[:, m0 : m0 + M_STRIP]

                # strip 0 exps in small leading pieces so the ACT stream
                # starts as soon as the first m chunks land; steady state
                # uses 2016-col pieces (fewer per-call overheads)
                if s == 0:
                    pieces = [range(0, 2), range(2, 4), range(4, 8)]
                else:
                    pieces = [range(0, 4), range(4, 8)]

                exp_t = exp_pool.tile([M_STRIP, N], f16, tag="exp")
                acc = stats_pool.tile([M_STRIP, len(pieces)], f32, tag="acc")

                for pi, piece in enumerate(pieces):
                    k = len(piece)
                    # 1 PSUM bank (512 f32) per 504-col chunk; chunks start on
                    # bank boundaries so PE writes never straddle one
                    ps = psum_pool.tile([M_STRIP, 512 * k], f32, tag="ps")
                    for cc, c in enumerate(piece):
                        nc.tensor.matmul(
                            ps[:, cc * 512 : cc * 512 + N_CHUNK],
                            q_l,
                            m_s[:, c * N_CHUNK : (c + 1) * N_CHUNK],
                            start=True,
                            stop=True,
                        )
                    # exp(logits + bias_m) PSUM->SBUF fp16 with fused
                    # per-partition row sum; 3D views skip the 8 pad cols/bank
                    e0 = piece[0] * N_CHUNK
                    nc.scalar.activation(
                        exp_t[:, e0 : e0 + k * N_CHUNK].rearrange(
                            "p (b c) -> p b c", b=k
                        ),
                        ps.rearrange("p (b c) -> p b c", b=k)[:, :, :N_CHUNK],
                        Exp,
                        bias=b_s[:, s : s + 1],
                        accum_out=acc[:, pi : pi + 1],
                    )

                ssum = stats_pool.tile([M_STRIP, 1], f32, tag="ssum")
                nc.vector.reduce_sum(ssum, acc, axis=mybir.AxisListType.X)
                rcp = stats_pool.tile([M_STRIP, 1], f32, tag="rcp")
                nc.vector.reciprocal(rcp, ssum)

                out_t = out_pool.tile([M_STRIP, N], f16, tag="out")
                if s == 0:
                    # quarters so the first bytes hit the DMA ring early
                    tsm_bounds = [0, 1008, 2016, 3024, N]
                    store_bounds = tsm_bounds
                elif s == N_STRIPS - 1:
                    # quartered scale + stores shorten the drain tail
                    tsm_bounds = [0, 1008, 2016, 3024, N]
                    store_bounds = tsm_bounds
                else:
                    tsm_bounds = [0, N]
                    store_bounds = [0, 2016, N]
                tsm_spans = dict(zip(tsm_bounds, tsm_bounds[1:]))
                for p0, p1 in zip(store_bounds, store_bounds[1:]):
                    if p0 in tsm_spans:
                        t1 = tsm_spans[p0]
                        nc.vector.tensor_scalar_mul(
                            out_t[:, p0:t1], exp_t[:, p0:t1], rcp
                        )
                    nc.sync.dma_start(
                        out=out_d[m0 : m0 + M_STRIP, p0:p1],
                        in_=out_t[:, p0:p1],
                    )

    nc.compile()
    return nc


def _get_nc():
    if "nc" not in _CACHE:
        _CACHE["nc"] = _build_nc()
    return _CACHE["nc"]


def _round_tf32(x: np.ndarray) -> np.ndarray:
    """Round f32 to 11-bit mantissa (tf32/f32r) with round-to-nearest."""
    xi = np.ascontiguousarray(x, dtype=np.float32).view(np.uint32)
    return ((xi + np.uint32(0x1000)) & np.uint32(0xFFFFE000)).view(np.float32)


def kernel(mk: np.ndarray, qk: np.ndarray) -> np.ndarray:
    from concourse import bass_utils

    mk = np.asarray(mk, dtype=np.float32).reshape(B, CK, N)
    qk = np.asarray(qk, dtype=np.float32).reshape(B, CK, N)
    a = np.einsum("bcn,bcn->bn", mk, mk)        # ||mk_n||^2, [B, N]
    cq = np.einsum("bcm,bcm->bm", qk, qk)       # ||qk_m||^2, [B, M]
    abar = a.mean(axis=1)                       # [B]

    in_maps = []
    for core in range(8):
        b, h = divmod(core, 2)
        m2 = np.empty((K_AUG, N), np.float32)
        m2[:CK] = mk[b]
        m2[CK] = a[b] - abar[b]
        m2 = _round_tf32(m2)

        q2 = np.empty((K_AUG, HALF), np.float32)
        q2[:CK] = 0.25 * qk[b, :, h * HALF : (h + 1) * HALF]
        q2[CK] = -0.125
        q2 = _round_tf32(q2)

        bias = (
            (-0.125 * (cq[b, h * HALF : (h + 1) * HALF] + abar[b]))
            .astype(np.float32)
            .reshape(N_STRIPS, M_STRIP)
            .T.copy()
        )
        in_maps.append({"q2": q2, "m2": m2, "bias": bias})

    res = bass_utils.run_bass_kernel_spmd(
        _get_nc(), in_maps, core_ids=list(range(8))
    )
    _CACHE["last_results"] = res

    out = np.empty((B, N, N), np.float32)
    for core in range(8):
        b, h = divmod(core, 2)
        out[b, :, h * HALF : (h + 1) * HALF] = (
            res.results[core]["out_c"].T.astype(np.float32)
        )
    return out
